# revision 27
# baseline (speedup 1.0000x reference)
"""Trainium2 Bass kernel for nn_ItemAutoencoder (LSTM autoencoder).

Model: x[B,T,D] -> relu(x @ in_W.T + in_b)            [B,T,64]
         -> LSTM(64->256) -> LSTM(256->256)            [B,T,256]
         -> z = h[:, -1]                               [B,256]
         -> repeat z over T -> LSTM(256->64) -> LSTM(64->64)
         -> out = d @ out_W.T + out_b                  [B,T,256]
B=1024, T=100, D=256.  Sharding: data-parallel, batch 128 per core x 8 cores.

Per-core design (v2 — latency-pipelined recurrence):
  - States kept transposed ([H, B] packed as [128, H] SBUF tiles) so they feed
    gate matmuls as lhsT directly; gates land in PSUM as [B, 4H].
  - Gate order host-permuted to [f, i, o, g]: PSUM bank A = (f, i), bank B =
    (o, g).  Activations split 4-way (sigm f / sigm i / tanh g / sigm o) so
    the c-update starts as soon as bank A lands.
  - L0's input-gate matmuls for step t+1 are pre-accumulated into a second
    PSUM buffer during slot t, so only the recurrent matmuls sit on the
    recurrence critical path.  L1's bias rides a K=1 ones-row matmul emitted
    at slot start (keeps the PE dense for HAM warmth).
  - All gate/tanh-c/h tensors are bf16 (DVE 2x mode); cell state c stays
    fp32.  Transposes run in bf16 (1 cyc/row).
  - Emission order per slot: dep-free matmul burst first (L0 rec, L1 all,
    L0-in prefetch), transposes last — PE never idles long enough for the
    HAM clock gate to re-throttle.
"""
import os
import numpy as np
from contextlib import ExitStack

import concourse.bass as bass
import concourse.tile as tile
from concourse import bacc, mybir
from concourse import bass_utils

F32 = mybir.dt.float32
BF16 = mybir.dt.bfloat16
import ml_dtypes
MM_DT = BF16
MM_NP = ml_dtypes.bfloat16
AF = mybir.ActivationFunctionType
TS = bass.ts

N_CORES = 8
B = 128            # per-core batch
T = 100
D = 256
H = 256            # encoder hidden
M = 64             # in-proj dim / decoder hidden
GE = 4 * H         # 1024
GD = 4 * M         # 256

_CACHE: dict = {}


def build_nc():
    nc = bacc.Bacc("TRN2", target_bir_lowering=False, debug=False)

    # ---- DRAM I/O -----------------------------------------------------------
    xT_d = nc.dram_tensor("xT", [2, 128, T * B], MM_DT, kind="ExternalInput")
    inWT_d = nc.dram_tensor("inWT", [2, 128, M], MM_DT, kind="ExternalInput")
    inb_d = nc.dram_tensor("inb", [M, 1], F32, kind="ExternalInput")
    w0in_d = nc.dram_tensor("w0in", [M + 1, GE], MM_DT, kind="ExternalInput")
    w0rec_d = nc.dram_tensor("w0rec", [2, 128, GE], MM_DT, kind="ExternalInput")
    w1in_d = nc.dram_tensor("w1in", [2, 128, GE], MM_DT, kind="ExternalInput")
    w1rec_d = nc.dram_tensor("w1rec", [2, 128, GE], MM_DT, kind="ExternalInput")
    b1_d = nc.dram_tensor("b1", [1, GE], MM_DT, kind="ExternalInput")
    dw0in_d = nc.dram_tensor("dw0in", [2, 128, GD], MM_DT, kind="ExternalInput")
    bd0_d = nc.dram_tensor("bd0", [1, GD], MM_DT, kind="ExternalInput")
    dw0rec_d = nc.dram_tensor("dw0rec", [M, GD], MM_DT, kind="ExternalInput")
    dw1in_d = nc.dram_tensor("dw1in", [M + 1, GD], MM_DT, kind="ExternalInput")
    dw1rec_d = nc.dram_tensor("dw1rec", [M, GD], MM_DT, kind="ExternalInput")
    wout_d = nc.dram_tensor("wout", [M + 1, D], MM_DT, kind="ExternalInput")
    identr_d = nc.dram_tensor("identr", [128, 128], MM_DT, kind="ExternalInput")
    ones1_d = nc.dram_tensor("ones1", [1, 128], MM_DT, kind="ExternalInput")
    out_d = nc.dram_tensor("out", [B, T * D], F32, kind="ExternalOutput")

    with tile.TileContext(nc) as tc, ExitStack() as ctx:
        P = bass.MemorySpace.PSUM
        wp = ctx.enter_context(tc.tile_pool(name="w", bufs=1))

        def wtile(dram_ap, shape, tag, dt=MM_DT):
            t_ = wp.tile(shape, dt, tag=tag)
            nc.sync.dma_start(t_[:], dram_ap)
            return t_

        # ---- persistent weights in SBUF ------------------------------------
        inWT = [wtile(inWT_d[kb, :, :], [128, M], f"inWT{kb}") for kb in range(2)]
        inb = wtile(inb_d[:], [M, 1], "inb", F32)
        w0in = wtile(w0in_d[:], [M + 1, GE], "w0in")
        w0rec = [wtile(w0rec_d[kb, :, :], [128, GE], f"w0rec{kb}") for kb in range(2)]
        w1in = [wtile(w1in_d[kb, :, :], [128, GE], f"w1in{kb}") for kb in range(2)]
        w1rec = [wtile(w1rec_d[kb, :, :], [128, GE], f"w1rec{kb}") for kb in range(2)]
        b1 = wtile(b1_d[:], [1, GE], "b1")
        dw0in = [wtile(dw0in_d[kb, :, :], [128, GD], f"dw0in{kb}") for kb in range(2)]
        bd0 = wtile(bd0_d[:], [1, GD], "bd0")
        dw0rec = wtile(dw0rec_d[:], [M, GD], "dw0rec")
        dw1in = wtile(dw1in_d[:], [M + 1, GD], "dw1in")
        dw1rec = wtile(dw1rec_d[:], [M, GD], "dw1rec")
        wout = wtile(wout_d[:], [M + 1, D], "wout")
        identr = wtile(identr_d[:], [128, 128], "identr")
        ones1 = wtile(ones1_d[:], [1, 128], "ones1")

        # in-proj output, transposed, with a ones row (row 64) for bias riding
        h0aug_h = nc.alloc_sbuf_tensor("h0aug", [M + 1, T * B], MM_DT)
        h0aug = h0aug_h.ap()
        nc.gpsimd.memset(h0aug[M : M + 1, :], 1.0)
        zz = wp.tile([M, 512], F32, tag="zz")
        nc.gpsimd.memset(zz[:], 0.0)

        # ============================= in-proj ==============================
        xpool = ctx.enter_context(tc.tile_pool(name="xc", bufs=4))
        with tc.tile_pool(name="psip", bufs=2, space=P) as psip:
            NG = T * B // 512  # 25
            for g in range(NG):
                xa = xpool.tile([128, 512], MM_DT, tag="xa")
                xb = xpool.tile([128, 512], MM_DT, tag="xb")
                nc.sync.dma_start(xa[:], xT_d[0, :, TS(g, 512)])
                nc.sync.dma_start(xb[:], xT_d[1, :, TS(g, 512)])
                ps = psip.tile([M, 512], F32)
                nc.tensor.matmul(ps[:], inWT[0][:], xa[:], start=True, stop=False)
                nc.tensor.matmul(ps[:], inWT[1][:], xb[:], start=False, stop=True)
                nc.vector.scalar_tensor_tensor(
                    h0aug[0:M, TS(g, 512)], ps[:], inb[:, 0:1], zz[:],
                    mybir.AluOpType.add, mybir.AluOpType.max,
                )

        # ============================= encoder ==============================
        gpool = ctx.enter_context(tc.tile_pool(name="g", bufs=2))
        spool = ctx.enter_context(tc.tile_pool(name="s", bufs=2))
        hpool = ctx.enter_context(tc.tile_pool(name="h", bufs=3))

        def new_state(tag, shape, dt, pool, zero=True):
            t_ = pool.tile(shape, dt, tag=tag)
            if zero:
                nc.gpsimd.memset(t_[:], 0.0)
            return t_

        st = {
            0: {
                "hT": new_state("h0T", [128, H], MM_DT, hpool),
                "c": new_state("c0", [128, H], F32, spool),
            },
            1: {
                "hT": new_state("h1T", [128, H], MM_DT, hpool),
                "c": new_state("c1", [128, H], F32, spool),
            },
        }

        with (
            tc.tile_pool(name="pg", bufs=3, space=P) as pgp,
            tc.tile_pool(name="ptr", bufs=2, space=P) as ptrp,
        ):
            # ---- L0 input-gate prefetch: accumulate W0in @ h0aug(t) into a
            # fresh pg0 buffer (start=True, stop=False); the recurrent matmuls
            # finish the accumulation in slot t.
            def l0_in_prefetch(t):
                ps = pgp.tile([128, GE], F32, tag="pg", name="pg_l0")
                for chunk in range(2):
                    ns = slice(chunk * 512, (chunk + 1) * 512)
                    nc.tensor.matmul(
                        ps[:, ns], h0aug[:, TS(t, 128)], w0in[:, ns],
                        start=True, stop=False,
                    )
                return ps

            def l0_rec_mms(t):
                s = st[0]
                ps = s["ps_next"]
                s["ps"] = ps
                hT = s["hT"]
                # bank A (cols 0:512) first so sigmoid(f) can start early
                for chunk in range(2):
                    ns = slice(chunk * 512, (chunk + 1) * 512)
                    for j, (lh, rh) in enumerate(
                        [(hT[:, 0:128], w0rec[0]), (hT[:, 128:256], w0rec[1])]
                    ):
                        nc.tensor.matmul(
                            ps[:, ns], lh, rh[:, ns],
                            start=False, stop=(chunk == 1 and j == 1),
                        )

            def l1_mms(t):
                s = st[1]
                ps = pgp.tile([128, GE], F32, tag="pg", name="pg_l1")
                s["ps"] = ps
                in_hT = s["in_hT"]
                hT = s["hT"]
                srcs = [
                    (ones1[:], b1),
                    (in_hT[:, 0:128], w1in[0]),
                    (in_hT[:, 128:256], w1in[1]),
                    (hT[:, 0:128], w1rec[0]),
                    (hT[:, 128:256], w1rec[1]),
                ]
                for chunk in range(2):
                    ns = slice(chunk * 512, (chunk + 1) * 512)
                    for j, (lh, rh) in enumerate(srcs):
                        nc.tensor.matmul(
                            ps[:, ns], lh, rh[:, ns],
                            start=(j == 0), stop=(j == len(srcs) - 1),
                        )

            # activations: 4-way split, bf16 outputs.  [f, i, o, g] layout:
            # f=0:H (bank A), i=H:2H (bank A), o=2H:3H (bank B), g=3H:4H (B).
            def act_f(l):
                s = st[l]
                gsb = gpool.tile([128, GE], MM_DT, tag=f"gsb{l}")
                s["gsb"] = gsb
                nc.scalar.activation(gsb[:, 0:H], s["ps"][:, 0:H], AF.Sigmoid)

            def act_i(l):
                s = st[l]
                nc.scalar.activation(
                    s["gsb"][:, H : 2 * H], s["ps"][:, H : 2 * H], AF.Sigmoid
                )

            def act_g(l):
                s = st[l]
                nc.scalar.activation(
                    s["gsb"][:, 3 * H :], s["ps"][:, 3 * H :], AF.Tanh
                )

            def act_o(l):
                s = st[l]
                nc.scalar.activation(
                    s["gsb"][:, 2 * H : 3 * H], s["ps"][:, 2 * H : 3 * H], AF.Sigmoid
                )

            def dve_fc(l):
                s = st[l]
                ctmp = spool.tile([128, H], F32, tag=f"ctmp{l}")
                nc.vector.tensor_mul(ctmp[:], s["gsb"][:, 0:H], s["c"][:])
                s["ctmp"] = ctmp

            def dve_u(l):
                s = st[l]
                u = spool.tile([128, H], MM_DT, tag=f"u{l}")
                nc.vector.tensor_mul(u[:], s["gsb"][:, H : 2 * H], s["gsb"][:, 3 * H :])
                s["u"] = u

            def dve_c(l):
                s = st[l]
                c_new = spool.tile([128, H], F32, tag=f"c{l}")
                nc.vector.tensor_add(c_new[:], s["ctmp"][:], s["u"][:])
                s["c"] = c_new

            def act_tanhc(l):
                s = st[l]
                tcs = spool.tile([128, H], MM_DT, tag=f"tc{l}")
                nc.scalar.activation(tcs[:], s["c"][:], AF.Tanh)
                s["tc"] = tcs

            def dve_h(l):
                s = st[l]
                hsb = spool.tile([128, H], MM_DT, tag=f"hsb{l}")
                nc.vector.tensor_mul(hsb[:], s["gsb"][:, 2 * H : 3 * H], s["tc"][:])
                s["h"] = hsb

            def pe_tr(l):
                s = st[l]
                ptr = ptrp.tile([128, H], MM_DT, tag="ptr")
                nc.tensor.transpose(ptr[:, 0:128], s["h"][:, 0:128], identr[:])
                nc.tensor.transpose(ptr[:, 128:256], s["h"][:, 128:256], identr[:])
                s["ptr"] = ptr

            def copy_h(l):
                s = st[l]
                hT_new = hpool.tile([128, H], MM_DT, tag=f"h{l}T")
                nc.vector.tensor_copy(hT_new[:], s["ptr"][:])
                s["hT"] = hT_new

            def enc_slot(work, prefetch_t):
                # work: list of layers, each (l, t); L0 first when present.
                has0 = any(l == 0 for l, _ in work)
                if has0:
                    l0_rec_mms([t for l, t in work if l == 0][0])
                for l, t in work:
                    if l == 1:
                        l1_mms(t)
                if prefetch_t is not None:
                    st[0]["ps_next"] = l0_in_prefetch(prefetch_t)
                for l, _ in work:
                    for fn in (act_f, dve_fc, act_i, act_g, dve_u, dve_c,
                               act_o, act_tanhc, dve_h):
                        fn(l)
                for l, _ in work:
                    pe_tr(l)
                for l, _ in work:
                    copy_h(l)

            # prologue: prefetch L0 inputs for t=0
            st[0]["ps_next"] = l0_in_prefetch(0)
            for t in range(T):
                work = [(0, t)]
                if t >= 1:
                    st[1]["in_hT"] = st[0]["hT"]
                    work.append((1, t - 1))
                enc_slot(work, t + 1 if t + 1 < T else None)
            st[1]["in_hT"] = st[0]["hT"]
            enc_slot([(1, T - 1)], None)

        zT = st[1]["hT"]  # [128, 256] bf16 = h1T(T-1)

        # ============================= decoder ==============================
        dst = {}
        with (
            tc.tile_pool(name="pd0", bufs=2, space=P) as pd0p,
            tc.tile_pool(name="pd1", bufs=1, space=P) as pd1p,
            tc.tile_pool(name="ptrd", bufs=2, space=P) as ptrdp,
            tc.tile_pool(name="pout", bufs=2, space=P) as poutp,
            tc.tile_pool(name="pxg", bufs=1, space=P) as pxgp,
        ):
            # xg0 = z @ dW0i.T + bd0  (constant over time)
            psx = pxgp.tile([128, GD], F32)
            nc.tensor.matmul(psx[:], ones1[:], bd0[:], start=True, stop=False)
            nc.tensor.matmul(psx[:], zT[:, 0:128], dw0in[0][:], start=False, stop=False)
            nc.tensor.matmul(psx[:], zT[:, 128:256], dw0in[1][:], start=False, stop=True)
            xg0 = wp.tile([128, GD], MM_DT, tag="xg0")
            nc.scalar.activation(xg0[:], psx[:], AF.Copy)

            def dstate(tag):
                t_ = nc.alloc_sbuf_tensor(tag, [M + 1, 128], MM_DT).ap()
                nc.gpsimd.memset(t_[:], 0.0)
                nc.gpsimd.memset(t_[M : M + 1, :], 1.0)
                return t_

            for l in range(2):
                dst[l] = {
                    "dT": [dstate(f"d{l}A"), dstate(f"d{l}B"), dstate(f"d{l}Z")],
                    "c": new_state(f"cd{l}", [128, M], F32, spool),
                    "prev": 2,
                }

            # dec L0 input prefetch: xg0 re-injected via identity matmul into
            # a fresh pd0 buffer (no deps — keeps PE warm, off critical path)
            def d0_in_prefetch():
                ps = pd0p.tile([128, GD], F32, tag="pd0")
                nc.tensor.matmul(ps[:], identr[:], xg0[:], start=True, stop=False)
                return ps

            def d0_rec_mms(t):
                s = dst[0]
                ps = s["ps_next"]
                s["ps"] = ps
                dT_prev = s["dT"][s["prev"]]
                nc.tensor.matmul(ps[:], dT_prev[0:M, :], dw0rec[:],
                                 start=False, stop=True)

            def d1_mms(t):
                s = dst[1]
                ps = pd1p.tile([128, GD], F32, tag="pd1")
                s["ps"] = ps
                d0T = dst[0]["in_dT"]
                dT_prev = s["dT"][s["prev"]]
                nc.tensor.matmul(ps[:], d0T[0 : M + 1, :], dw1in[:],
                                 start=True, stop=False)
                nc.tensor.matmul(ps[:], dT_prev[0:M, :], dw1rec[:],
                                 start=False, stop=True)

            def dact_fio(l):
                s = dst[l]
                gsb = gpool.tile([128, GD], MM_DT, tag=f"dgsb{l}")
                s["gsb"] = gsb
                nc.scalar.activation(gsb[:, 0 : 3 * M], s["ps"][:, 0 : 3 * M],
                                     AF.Sigmoid)

            def dact_g(l):
                s = dst[l]
                nc.scalar.activation(s["gsb"][:, 3 * M :], s["ps"][:, 3 * M :],
                                     AF.Tanh)

            def ddve_fc(l):
                s = dst[l]
                ctmp = spool.tile([128, M], F32, tag=f"dctmp{l}")
                nc.vector.tensor_mul(ctmp[:], s["gsb"][:, 0:M], s["c"][:])
                s["ctmp"] = ctmp

            def ddve_u(l):
                s = dst[l]
                u = spool.tile([128, M], MM_DT, tag=f"du{l}")
                nc.vector.tensor_mul(u[:], s["gsb"][:, M : 2 * M],
                                     s["gsb"][:, 3 * M :])
                s["u"] = u

            def ddve_c(l):
                s = dst[l]
                c_new = spool.tile([128, M], F32, tag=f"dc{l}")
                nc.vector.tensor_add(c_new[:], s["ctmp"][:], s["u"][:])
                s["c"] = c_new

            def dact_tanhc(l):
                s = dst[l]
                tcs = spool.tile([128, M], MM_DT, tag=f"dtc{l}")
                nc.scalar.activation(tcs[:], s["c"][:], AF.Tanh)
                s["tc"] = tcs

            def ddve_h(l):
                s = dst[l]
                hsb = spool.tile([128, M], MM_DT, tag=f"dhsb{l}")
                nc.vector.tensor_mul(hsb[:], s["gsb"][:, 2 * M : 3 * M], s["tc"][:])
                s["h"] = hsb

            def dpe_tr(l):
                s = dst[l]
                ptr = ptrdp.tile([M, 128], MM_DT, tag="ptrd")
                nc.tensor.transpose(ptr[:], s["h"][:], identr[:])
                s["ptr"] = ptr

            def dcopy_h(l, t):
                s = dst[l]
                dT_new = s["dT"][t % 2]
                if l == 0:
                    nc.scalar.activation(dT_new[0:M, :], s["ptr"][:], AF.Copy)
                else:
                    nc.vector.tensor_copy(dT_new[0:M, :], s["ptr"][:])
                s["prev"] = t % 2

            ochunk = {"tile": None}

            def outproj_mm(t):
                d1T = dst[1]["dT"][t % 2]
                ps = poutp.tile([128, D], F32, tag="pout")
                nc.tensor.matmul(ps[:], d1T[0 : M + 1, :], wout[:],
                                 start=True, stop=True)
                return ps

            def outproj_copy(t, ps):
                if t % 10 == 0:
                    ochunk["tile"] = spool.tile(
                        [128, 10 * D], F32, tag="ochunk", name="ochunk"
                    )
                dest = ochunk["tile"][:, TS(t % 10, D)]
                nc.vector.tensor_copy(dest, ps[:])
                if t % 10 == 9:
                    nc.sync.dma_start(out_d[:, TS(t // 10, 10 * D)], ochunk["tile"][:])

            def dec_slot(work, op_t, prefetch):
                has0 = any(l == 0 for l, _ in work)
                if has0:
                    d0_rec_mms([t for l, t in work if l == 0][0])
                for l, t in work:
                    if l == 1:
                        d1_mms(t)
                ps_out = outproj_mm(op_t) if op_t is not None else None
                if prefetch:
                    dst[0]["ps_next"] = d0_in_prefetch()
                for l, _ in work:
                    for fn in (dact_fio, ddve_fc, dact_g, ddve_u, ddve_c,
                               dact_tanhc, ddve_h):
                        fn(l)
                if op_t is not None:
                    outproj_copy(op_t, ps_out)
                for l, _ in work:
                    dpe_tr(l)
                for l, t in work:
                    dcopy_h(l, t)

            dst[0]["ps_next"] = d0_in_prefetch()
            for t in range(T):
                work = [(0, t)]
                if t >= 1:
                    dst[0]["in_dT"] = dst[0]["dT"][(t - 1) % 2]
                    work.append((1, t - 1))
                op_t = t - 2 if t >= 2 else None
                dec_slot(work, op_t, prefetch=(t + 1 < T))
            dst[0]["in_dT"] = dst[0]["dT"][(T - 1) % 2]
            dec_slot([(1, T - 1)], T - 2, prefetch=False)
            dec_slot([], T - 1, prefetch=False)

    nc.compile()
    return nc


# ----------------------------------------------------------------------------
# host-side wrapper
# ----------------------------------------------------------------------------

def _perm(n):
    """pytorch gate order i,f,g,o (blocks of n) -> [f, i, o, g]."""
    idx = np.arange(4 * n).reshape(4, n)
    return np.concatenate([idx[1], idx[0], idx[3], idx[2]])


def _prep_core_inputs(inputs, core):
    f = np.float32
    pe = _perm(H)
    pd = _perm(M)
    x = inputs["x"][core * B : (core + 1) * B]          # [128, 100, 256]
    xT = np.ascontiguousarray(x.transpose(2, 1, 0)).reshape(2, 128, T * B)

    w0in = np.concatenate(
        [inputs["eW0i"].T[:, pe], (inputs["eb0i"] + inputs["eb0h"])[None, pe]], 0
    )
    w0rec = inputs["eW0h"].T[:, pe].reshape(2, 128, GE)
    w1in = inputs["eW1i"].T[:, pe].reshape(2, 128, GE)
    w1rec = inputs["eW1h"].T[:, pe].reshape(2, 128, GE)
    b1 = (inputs["eb1i"] + inputs["eb1h"])[None, pe]
    dw0in = inputs["dW0i"].T[:, pd].reshape(2, 128, GD)
    bd0 = (inputs["db0i"] + inputs["db0h"])[None, pd]
    dw0rec = inputs["dW0h"].T[:, pd]
    dw1in = np.concatenate(
        [inputs["dW1i"].T[:, pd], (inputs["db1i"] + inputs["db1h"])[None, pd]], 0
    )
    dw1rec = inputs["dW1h"].T[:, pd]
    wout = np.concatenate([inputs["out_W"].T, inputs["out_b"][None, :]], 0)

    g = MM_NP
    return {
        "xT": np.ascontiguousarray(xT, dtype=g),
        "inWT": np.ascontiguousarray(inputs["in_W"].T.reshape(2, 128, M), dtype=g),
        "inb": np.ascontiguousarray(inputs["in_b"][:, None], dtype=f),
        "w0in": np.ascontiguousarray(w0in, dtype=g),
        "w0rec": np.ascontiguousarray(w0rec, dtype=g),
        "w1in": np.ascontiguousarray(w1in, dtype=g),
        "w1rec": np.ascontiguousarray(w1rec, dtype=g),
        "b1": np.ascontiguousarray(b1, dtype=g),
        "dw0in": np.ascontiguousarray(dw0in, dtype=g),
        "bd0": np.ascontiguousarray(bd0, dtype=g),
        "dw0rec": np.ascontiguousarray(dw0rec, dtype=g),
        "dw1in": np.ascontiguousarray(dw1in, dtype=g),
        "dw1rec": np.ascontiguousarray(dw1rec, dtype=g),
        "wout": np.ascontiguousarray(wout, dtype=g),
        "identr": np.eye(128).astype(g),
        "ones1": np.ones((1, 128), dtype=g),
    }


def kernel(**inputs):
    inputs = {k: np.asarray(v, dtype=np.float32) for k, v in inputs.items()}
    if "nc" not in _CACHE:
        _CACHE["nc"] = build_nc()
    nc = _CACHE["nc"]
    in_maps = [_prep_core_inputs(inputs, c) for c in range(N_CORES)]
    trace = bool(int(os.environ.get("KERNEL_TRACE", "0")))
    res = bass_utils.run_bass_kernel_spmd(
        nc,
        in_maps,
        core_ids=list(range(N_CORES)),
        trace=trace,
        tmpdir=os.environ.get("KERNEL_TRACE_DIR") or None,
    )
    _CACHE["last_result"] = res
    out = np.concatenate(
        [res.results[c]["out"].reshape(B, T, D) for c in range(N_CORES)], axis=0
    )
    return out


# revision 28
# speedup vs baseline: 1.2104x; 1.2104x over previous
"""Trainium2 Bass kernel for nn_ItemAutoencoder (LSTM autoencoder).

Model: x[B,T,D] -> relu(x @ in_W.T + in_b)            [B,T,64]
         -> LSTM(64->256) -> LSTM(256->256)            [B,T,256]
         -> z = h[:, -1]                               [B,256]
         -> repeat z over T -> LSTM(256->64) -> LSTM(64->64)
         -> out = d @ out_W.T + out_b                  [B,T,256]
B=1024, T=100, D=256.  Sharding: data-parallel, batch 128 per core x 8 cores.

Per-core design (v2 — latency-pipelined recurrence):
  - States kept transposed ([H, B] packed as [128, H] SBUF tiles) so they feed
    gate matmuls as lhsT directly; gates land in PSUM as [B, 4H].
  - Gate order host-permuted to [f, i, o, g]: PSUM bank A = (f, i), bank B =
    (o, g).  Activations split 4-way (sigm f / sigm i / tanh g / sigm o) so
    the c-update starts as soon as bank A lands.
  - L0's input-gate matmuls for step t+1 are pre-accumulated into a second
    PSUM buffer during slot t, so only the recurrent matmuls sit on the
    recurrence critical path.  L1's bias rides a K=1 ones-row matmul emitted
    at slot start (keeps the PE dense for HAM warmth).
  - All gate/tanh-c/h tensors are bf16 (DVE 2x mode); cell state c stays
    fp32.  Transposes run in bf16 (1 cyc/row).
  - Emission order per slot: dep-free matmul burst first (L0 rec, L1 all,
    L0-in prefetch), transposes last — PE never idles long enough for the
    HAM clock gate to re-throttle.
"""
import os
import numpy as np
from contextlib import ExitStack

import concourse.bass as bass
import concourse.tile as tile
from concourse import bacc, mybir
from concourse import bass_utils

F32 = mybir.dt.float32
BF16 = mybir.dt.bfloat16
import ml_dtypes
MM_DT = BF16
MM_NP = ml_dtypes.bfloat16
AF = mybir.ActivationFunctionType
TS = bass.ts

N_CORES = 8
B = 128            # per-core batch
T = 100
D = 256
H = 256            # encoder hidden
M = 64             # in-proj dim / decoder hidden
GE = 4 * H         # 1024
GD = 4 * M         # 256

_CACHE: dict = {}


def build_nc():
    nc = bacc.Bacc("TRN2", target_bir_lowering=False, debug=False)

    # ---- DRAM I/O -----------------------------------------------------------
    xT_d = nc.dram_tensor("xT", [2, 128, T * B], MM_DT, kind="ExternalInput")
    inWT_d = nc.dram_tensor("inWT", [2, 128, M], MM_DT, kind="ExternalInput")
    inb_d = nc.dram_tensor("inb", [M, 1], F32, kind="ExternalInput")
    w0in_d = nc.dram_tensor("w0in", [M + 1, GE], MM_DT, kind="ExternalInput")
    w0rec_d = nc.dram_tensor("w0rec", [2, 128, GE], MM_DT, kind="ExternalInput")
    w1in_d = nc.dram_tensor("w1in", [2, 128, GE], MM_DT, kind="ExternalInput")
    w1rec_d = nc.dram_tensor("w1rec", [2, 128, GE], MM_DT, kind="ExternalInput")
    b1_d = nc.dram_tensor("b1", [1, GE], MM_DT, kind="ExternalInput")
    dw0in_d = nc.dram_tensor("dw0in", [2, 128, GD], MM_DT, kind="ExternalInput")
    bd0_d = nc.dram_tensor("bd0", [1, GD], MM_DT, kind="ExternalInput")
    dw0rec_d = nc.dram_tensor("dw0rec", [M, GD], MM_DT, kind="ExternalInput")
    dw1in_d = nc.dram_tensor("dw1in", [M + 1, GD], MM_DT, kind="ExternalInput")
    dw1rec_d = nc.dram_tensor("dw1rec", [M, GD], MM_DT, kind="ExternalInput")
    wout_d = nc.dram_tensor("wout", [M + 1, D], MM_DT, kind="ExternalInput")
    identr_d = nc.dram_tensor("identr", [128, 128], MM_DT, kind="ExternalInput")
    ones1_d = nc.dram_tensor("ones1", [1, 128], MM_DT, kind="ExternalInput")
    out_d = nc.dram_tensor("out", [B, T * D], F32, kind="ExternalOutput")

    with tile.TileContext(nc) as tc, ExitStack() as ctx:
        P = bass.MemorySpace.PSUM
        wp = ctx.enter_context(tc.tile_pool(name="w", bufs=1))

        def wtile(dram_ap, shape, tag, dt=MM_DT):
            t_ = wp.tile(shape, dt, tag=tag)
            nc.sync.dma_start(t_[:], dram_ap)
            return t_

        # ---- persistent weights in SBUF ------------------------------------
        inWT = [wtile(inWT_d[kb, :, :], [128, M], f"inWT{kb}") for kb in range(2)]
        inb = wtile(inb_d[:], [M, 1], "inb", F32)
        w0in = wtile(w0in_d[:], [M + 1, GE], "w0in")
        w0rec = [wtile(w0rec_d[kb, :, :], [128, GE], f"w0rec{kb}") for kb in range(2)]
        w1in = [wtile(w1in_d[kb, :, :], [128, GE], f"w1in{kb}") for kb in range(2)]
        w1rec = [wtile(w1rec_d[kb, :, :], [128, GE], f"w1rec{kb}") for kb in range(2)]
        b1 = wtile(b1_d[:], [1, GE], "b1")
        dw0in = [wtile(dw0in_d[kb, :, :], [128, GD], f"dw0in{kb}") for kb in range(2)]
        bd0 = wtile(bd0_d[:], [1, GD], "bd0")
        dw0rec = wtile(dw0rec_d[:], [M, GD], "dw0rec")
        dw1in = wtile(dw1in_d[:], [M + 1, GD], "dw1in")
        dw1rec = wtile(dw1rec_d[:], [M, GD], "dw1rec")
        wout = wtile(wout_d[:], [M + 1, D], "wout")
        identr = wtile(identr_d[:], [128, 128], "identr")
        ones1 = wtile(ones1_d[:], [1, 128], "ones1")

        # in-proj output, transposed, with a ones row (row 64) for bias riding
        h0aug_h = nc.alloc_sbuf_tensor("h0aug", [M + 1, T * B], MM_DT)
        h0aug = h0aug_h.ap()
        nc.gpsimd.memset(h0aug[M : M + 1, :], 1.0)

        # ============================= in-proj ==============================
        xpool = ctx.enter_context(tc.tile_pool(name="xc", bufs=4))
        with tc.tile_pool(name="psip", bufs=2, space=P) as psip:
            NG = T * B // 512  # 25
            for g in range(NG):
                xa = xpool.tile([128, 512], MM_DT, tag="xa")
                xb = xpool.tile([128, 512], MM_DT, tag="xb")
                nc.sync.dma_start(xa[:], xT_d[0, :, TS(g, 512)])
                nc.sync.dma_start(xb[:], xT_d[1, :, TS(g, 512)])
                ps = psip.tile([M, 512], F32)
                nc.tensor.matmul(ps[:], inWT[0][:], xa[:], start=True, stop=False)
                nc.tensor.matmul(ps[:], inWT[1][:], xb[:], start=False, stop=True)
                nc.scalar.activation(
                    h0aug[0:M, TS(g, 512)], ps[:], AF.Relu, bias=inb[:, 0:1]
                )

        # ============================= encoder ==============================
        gpool = ctx.enter_context(tc.tile_pool(name="g", bufs=2))
        spool = ctx.enter_context(tc.tile_pool(name="s", bufs=2))
        hpool = ctx.enter_context(tc.tile_pool(name="h", bufs=3))

        def new_state(tag, shape, dt, pool, zero=True):
            t_ = pool.tile(shape, dt, tag=tag)
            if zero:
                nc.gpsimd.memset(t_[:], 0.0)
            return t_

        st = {
            0: {
                "hT": new_state("h0T", [128, H], MM_DT, hpool),
                "c": new_state("c0", [128, H], F32, spool),
            },
            1: {
                "hT": new_state("h1T", [128, H], MM_DT, hpool),
                "c": new_state("c1", [128, H], F32, spool),
            },
        }

        with (
            tc.tile_pool(name="pg", bufs=3, space=P) as pgp,
            tc.tile_pool(name="ptr", bufs=2, space=P) as ptrp,
        ):
            # ---- L0 input-gate prefetch: accumulate W0in @ h0aug(t) into a
            # fresh pg0 buffer (start=True, stop=False); the recurrent matmuls
            # finish the accumulation in slot t.
            def l0_in_prefetch(t):
                ps = pgp.tile([128, GE], F32, tag="pg", name="pg_l0")
                for chunk in range(2):
                    ns = slice(chunk * 512, (chunk + 1) * 512)
                    nc.tensor.matmul(
                        ps[:, ns], h0aug[:, TS(t, 128)], w0in[:, ns],
                        start=True, stop=False,
                    )
                return ps

            def l0_rec_mms(t):
                s = st[0]
                ps = s["ps_next"]
                s["ps"] = ps
                hT = s["hT"]
                # bank A (cols 0:512) first so sigmoid(f) can start early
                for chunk in range(2):
                    ns = slice(chunk * 512, (chunk + 1) * 512)
                    for j, (lh, rh) in enumerate(
                        [(hT[:, 0:128], w0rec[0]), (hT[:, 128:256], w0rec[1])]
                    ):
                        nc.tensor.matmul(
                            ps[:, ns], lh, rh[:, ns],
                            start=False, stop=(chunk == 1 and j == 1),
                        )

            def l1_mms(t):
                s = st[1]
                ps = pgp.tile([128, GE], F32, tag="pg", name="pg_l1")
                s["ps"] = ps
                in_hT = s["in_hT"]
                hT = s["hT"]
                srcs = [
                    (ones1[:], b1),
                    (in_hT[:, 0:128], w1in[0]),
                    (in_hT[:, 128:256], w1in[1]),
                    (hT[:, 0:128], w1rec[0]),
                    (hT[:, 128:256], w1rec[1]),
                ]
                for chunk in range(2):
                    ns = slice(chunk * 512, (chunk + 1) * 512)
                    for j, (lh, rh) in enumerate(srcs):
                        nc.tensor.matmul(
                            ps[:, ns], lh, rh[:, ns],
                            start=(j == 0), stop=(j == len(srcs) - 1),
                        )

            # activations: 4-way split, bf16 outputs.  [f, i, o, g] layout:
            # f=0:H (bank A), i=H:2H (bank A), o=2H:3H (bank B), g=3H:4H (B).
            def act_f(l):
                s = st[l]
                gsb = gpool.tile([128, GE], MM_DT, tag=f"gsb{l}")
                s["gsb"] = gsb
                nc.scalar.activation(gsb[:, 0:H], s["ps"][:, 0:H], AF.Sigmoid)

            def act_i(l):
                s = st[l]
                nc.scalar.activation(
                    s["gsb"][:, H : 2 * H], s["ps"][:, H : 2 * H], AF.Sigmoid
                )

            def act_g(l):
                s = st[l]
                nc.scalar.activation(
                    s["gsb"][:, 3 * H :], s["ps"][:, 3 * H :], AF.Tanh
                )

            def act_o(l):
                s = st[l]
                nc.scalar.activation(
                    s["gsb"][:, 2 * H : 3 * H], s["ps"][:, 2 * H : 3 * H], AF.Sigmoid
                )

            def dve_fc(l):
                s = st[l]
                ctmp = spool.tile([128, H], F32, tag=f"ctmp{l}")
                nc.vector.tensor_mul(ctmp[:], s["gsb"][:, 0:H], s["c"][:])
                s["ctmp"] = ctmp

            def dve_u(l):
                s = st[l]
                u = spool.tile([128, H], MM_DT, tag=f"u{l}")
                nc.vector.tensor_mul(u[:], s["gsb"][:, H : 2 * H], s["gsb"][:, 3 * H :])
                s["u"] = u

            def dve_c(l):
                s = st[l]
                c_new = spool.tile([128, H], F32, tag=f"c{l}")
                nc.vector.tensor_add(c_new[:], s["ctmp"][:], s["u"][:])
                s["c"] = c_new

            def act_tanhc(l):
                s = st[l]
                tcs = spool.tile([128, H], MM_DT, tag=f"tc{l}")
                nc.scalar.activation(tcs[:], s["c"][:], AF.Tanh)
                s["tc"] = tcs

            def dve_h(l):
                s = st[l]
                hsb = spool.tile([128, H], MM_DT, tag=f"hsb{l}")
                nc.vector.tensor_mul(hsb[:], s["gsb"][:, 2 * H : 3 * H], s["tc"][:])
                s["h"] = hsb

            def pe_tr(l):
                s = st[l]
                ptr = ptrp.tile([128, H], MM_DT, tag="ptr")
                nc.tensor.transpose(ptr[:, 0:128], s["h"][:, 0:128], identr[:])
                nc.tensor.transpose(ptr[:, 128:256], s["h"][:, 128:256], identr[:])
                s["ptr"] = ptr

            def copy_h(l):
                s = st[l]
                hT_new = hpool.tile([128, H], MM_DT, tag=f"h{l}T")
                nc.vector.tensor_copy(hT_new[:], s["ptr"][:])
                s["hT"] = hT_new

            def enc_slot(work, prefetch_t):
                # work: list of layers, each (l, t); L0 first when present.
                has0 = any(l == 0 for l, _ in work)
                if has0:
                    l0_rec_mms([t for l, t in work if l == 0][0])
                for l, t in work:
                    if l == 1:
                        l1_mms(t)
                if prefetch_t is not None:
                    st[0]["ps_next"] = l0_in_prefetch(prefetch_t)
                for l, _ in work:
                    for fn in (act_f, dve_fc, act_i, act_g, dve_u, dve_c,
                               act_o, act_tanhc, dve_h):
                        fn(l)
                for l, _ in work:
                    pe_tr(l)
                for l, _ in work:
                    copy_h(l)

            # prologue: prefetch L0 inputs for t=0
            st[0]["ps_next"] = l0_in_prefetch(0)
            for t in range(T):
                work = [(0, t)]
                if t >= 1:
                    st[1]["in_hT"] = st[0]["hT"]
                    work.append((1, t - 1))
                enc_slot(work, t + 1 if t + 1 < T else None)
            st[1]["in_hT"] = st[0]["hT"]
            enc_slot([(1, T - 1)], None)

        zT = st[1]["hT"]  # [128, 256] bf16 = h1T(T-1)

        # ============================= decoder ==============================
        dst = {}
        with (
            tc.tile_pool(name="pd0", bufs=2, space=P) as pd0p,
            tc.tile_pool(name="pd1", bufs=1, space=P) as pd1p,
            tc.tile_pool(name="ptrd", bufs=2, space=P) as ptrdp,
            tc.tile_pool(name="pout", bufs=2, space=P) as poutp,
            tc.tile_pool(name="pxg", bufs=1, space=P) as pxgp,
        ):
            # xg0 = z @ dW0i.T + bd0  (constant over time)
            psx = pxgp.tile([128, GD], F32)
            nc.tensor.matmul(psx[:], ones1[:], bd0[:], start=True, stop=False)
            nc.tensor.matmul(psx[:], zT[:, 0:128], dw0in[0][:], start=False, stop=False)
            nc.tensor.matmul(psx[:], zT[:, 128:256], dw0in[1][:], start=False, stop=True)
            xg0 = wp.tile([128, GD], MM_DT, tag="xg0")
            nc.scalar.activation(xg0[:], psx[:], AF.Copy)

            def dstate(tag):
                t_ = nc.alloc_sbuf_tensor(tag, [M + 1, 128], MM_DT).ap()
                nc.gpsimd.memset(t_[:], 0.0)
                nc.gpsimd.memset(t_[M : M + 1, :], 1.0)
                return t_

            for l in range(2):
                dst[l] = {
                    "dT": [dstate(f"d{l}A"), dstate(f"d{l}B"), dstate(f"d{l}Z")],
                    "c": new_state(f"cd{l}", [128, M], F32, spool),
                    "prev": 2,
                }

            # dec L0 input prefetch: xg0 re-injected via identity matmul into
            # a fresh pd0 buffer (no deps — keeps PE warm, off critical path)
            def d0_in_prefetch():
                ps = pd0p.tile([128, GD], F32, tag="pd0")
                nc.tensor.matmul(ps[:], identr[:], xg0[:], start=True, stop=False)
                return ps

            def d0_rec_mms(t):
                s = dst[0]
                ps = s["ps_next"]
                s["ps"] = ps
                dT_prev = s["dT"][s["prev"]]
                nc.tensor.matmul(ps[:], dT_prev[0:M, :], dw0rec[:],
                                 start=False, stop=True)

            def d1_mms(t):
                s = dst[1]
                ps = pd1p.tile([128, GD], F32, tag="pd1")
                s["ps"] = ps
                d0T = dst[0]["in_dT"]
                dT_prev = s["dT"][s["prev"]]
                nc.tensor.matmul(ps[:], d0T[0 : M + 1, :], dw1in[:],
                                 start=True, stop=False)
                nc.tensor.matmul(ps[:], dT_prev[0:M, :], dw1rec[:],
                                 start=False, stop=True)

            def dact_fio(l):
                s = dst[l]
                gsb = gpool.tile([128, GD], MM_DT, tag=f"dgsb{l}")
                s["gsb"] = gsb
                nc.scalar.activation(gsb[:, 0 : 3 * M], s["ps"][:, 0 : 3 * M],
                                     AF.Sigmoid)

            def dact_g(l):
                s = dst[l]
                nc.scalar.activation(s["gsb"][:, 3 * M :], s["ps"][:, 3 * M :],
                                     AF.Tanh)

            def ddve_fc(l):
                s = dst[l]
                ctmp = spool.tile([128, M], F32, tag=f"dctmp{l}")
                nc.vector.tensor_mul(ctmp[:], s["gsb"][:, 0:M], s["c"][:])
                s["ctmp"] = ctmp

            def ddve_u(l):
                s = dst[l]
                u = spool.tile([128, M], MM_DT, tag=f"du{l}")
                nc.vector.tensor_mul(u[:], s["gsb"][:, M : 2 * M],
                                     s["gsb"][:, 3 * M :])
                s["u"] = u

            def ddve_c(l):
                s = dst[l]
                c_new = spool.tile([128, M], F32, tag=f"dc{l}")
                nc.vector.tensor_add(c_new[:], s["ctmp"][:], s["u"][:])
                s["c"] = c_new

            def dact_tanhc(l):
                s = dst[l]
                tcs = spool.tile([128, M], MM_DT, tag=f"dtc{l}")
                nc.scalar.activation(tcs[:], s["c"][:], AF.Tanh)
                s["tc"] = tcs

            def ddve_h(l):
                s = dst[l]
                hsb = spool.tile([128, M], MM_DT, tag=f"dhsb{l}")
                nc.vector.tensor_mul(hsb[:], s["gsb"][:, 2 * M : 3 * M], s["tc"][:])
                s["h"] = hsb

            def dpe_tr(l):
                s = dst[l]
                ptr = ptrdp.tile([M, 128], MM_DT, tag="ptrd")
                nc.tensor.transpose(ptr[:], s["h"][:], identr[:])
                s["ptr"] = ptr

            def dcopy_h(l, t):
                s = dst[l]
                dT_new = s["dT"][t % 2]
                if l == 0:
                    nc.scalar.activation(dT_new[0:M, :], s["ptr"][:], AF.Copy)
                else:
                    nc.vector.tensor_copy(dT_new[0:M, :], s["ptr"][:])
                s["prev"] = t % 2

            ochunk = {"tile": None}

            def outproj_mm(t):
                d1T = dst[1]["dT"][t % 2]
                ps = poutp.tile([128, D], F32, tag="pout")
                nc.tensor.matmul(ps[:], d1T[0 : M + 1, :], wout[:],
                                 start=True, stop=True)
                return ps

            def outproj_copy(t, ps):
                if t % 10 == 0:
                    ochunk["tile"] = spool.tile(
                        [128, 10 * D], F32, tag="ochunk", name="ochunk"
                    )
                dest = ochunk["tile"][:, TS(t % 10, D)]
                if t % 2 == 0:
                    nc.scalar.activation(dest, ps[:], AF.Copy)
                else:
                    nc.vector.tensor_copy(dest, ps[:])
                if t % 10 == 9:
                    nc.sync.dma_start(out_d[:, TS(t // 10, 10 * D)], ochunk["tile"][:])

            def dec_slot(work, op_t, prefetch):
                has0 = any(l == 0 for l, _ in work)
                if has0:
                    d0_rec_mms([t for l, t in work if l == 0][0])
                for l, t in work:
                    if l == 1:
                        d1_mms(t)
                ps_out = outproj_mm(op_t) if op_t is not None else None
                if prefetch:
                    dst[0]["ps_next"] = d0_in_prefetch()
                for l, _ in work:
                    for fn in (dact_fio, ddve_fc, dact_g, ddve_u, ddve_c,
                               dact_tanhc, ddve_h):
                        fn(l)
                if op_t is not None:
                    outproj_copy(op_t, ps_out)
                for l, _ in work:
                    dpe_tr(l)
                for l, t in work:
                    dcopy_h(l, t)

            dst[0]["ps_next"] = d0_in_prefetch()
            for t in range(T):
                work = [(0, t)]
                if t >= 1:
                    dst[0]["in_dT"] = dst[0]["dT"][(t - 1) % 2]
                    work.append((1, t - 1))
                op_t = t - 2 if t >= 2 else None
                dec_slot(work, op_t, prefetch=(t + 1 < T))
            dst[0]["in_dT"] = dst[0]["dT"][(T - 1) % 2]
            dec_slot([(1, T - 1)], T - 2, prefetch=False)
            dec_slot([], T - 1, prefetch=False)

    nc.compile()
    return nc


# ----------------------------------------------------------------------------
# host-side wrapper
# ----------------------------------------------------------------------------

def _perm(n):
    """pytorch gate order i,f,g,o (blocks of n) -> [f, i, o, g]."""
    idx = np.arange(4 * n).reshape(4, n)
    return np.concatenate([idx[1], idx[0], idx[3], idx[2]])


def _prep_core_inputs(inputs, core):
    f = np.float32
    pe = _perm(H)
    pd = _perm(M)
    x = inputs["x"][core * B : (core + 1) * B]          # [128, 100, 256]
    xT = np.ascontiguousarray(x.transpose(2, 1, 0)).reshape(2, 128, T * B)

    w0in = np.concatenate(
        [inputs["eW0i"].T[:, pe], (inputs["eb0i"] + inputs["eb0h"])[None, pe]], 0
    )
    w0rec = inputs["eW0h"].T[:, pe].reshape(2, 128, GE)
    w1in = inputs["eW1i"].T[:, pe].reshape(2, 128, GE)
    w1rec = inputs["eW1h"].T[:, pe].reshape(2, 128, GE)
    b1 = (inputs["eb1i"] + inputs["eb1h"])[None, pe]
    dw0in = inputs["dW0i"].T[:, pd].reshape(2, 128, GD)
    bd0 = (inputs["db0i"] + inputs["db0h"])[None, pd]
    dw0rec = inputs["dW0h"].T[:, pd]
    dw1in = np.concatenate(
        [inputs["dW1i"].T[:, pd], (inputs["db1i"] + inputs["db1h"])[None, pd]], 0
    )
    dw1rec = inputs["dW1h"].T[:, pd]
    wout = np.concatenate([inputs["out_W"].T, inputs["out_b"][None, :]], 0)

    g = MM_NP
    return {
        "xT": np.ascontiguousarray(xT, dtype=g),
        "inWT": np.ascontiguousarray(inputs["in_W"].T.reshape(2, 128, M), dtype=g),
        "inb": np.ascontiguousarray(inputs["in_b"][:, None], dtype=f),
        "w0in": np.ascontiguousarray(w0in, dtype=g),
        "w0rec": np.ascontiguousarray(w0rec, dtype=g),
        "w1in": np.ascontiguousarray(w1in, dtype=g),
        "w1rec": np.ascontiguousarray(w1rec, dtype=g),
        "b1": np.ascontiguousarray(b1, dtype=g),
        "dw0in": np.ascontiguousarray(dw0in, dtype=g),
        "bd0": np.ascontiguousarray(bd0, dtype=g),
        "dw0rec": np.ascontiguousarray(dw0rec, dtype=g),
        "dw1in": np.ascontiguousarray(dw1in, dtype=g),
        "dw1rec": np.ascontiguousarray(dw1rec, dtype=g),
        "wout": np.ascontiguousarray(wout, dtype=g),
        "identr": np.eye(128).astype(g),
        "ones1": np.ones((1, 128), dtype=g),
    }


def kernel(**inputs):
    inputs = {k: np.asarray(v, dtype=np.float32) for k, v in inputs.items()}
    if "nc" not in _CACHE:
        _CACHE["nc"] = build_nc()
    nc = _CACHE["nc"]
    in_maps = [_prep_core_inputs(inputs, c) for c in range(N_CORES)]
    trace = bool(int(os.environ.get("KERNEL_TRACE", "0")))
    res = bass_utils.run_bass_kernel_spmd(
        nc,
        in_maps,
        core_ids=list(range(N_CORES)),
        trace=trace,
        tmpdir=os.environ.get("KERNEL_TRACE_DIR") or None,
    )
    _CACHE["last_result"] = res
    out = np.concatenate(
        [res.results[c]["out"].reshape(B, T, D) for c in range(N_CORES)], axis=0
    )
    return out
